# revision 1
# baseline (speedup 1.0000x reference)
"""Trainium2 Bass kernel: mixture-of-Gaussians mean log-likelihood.

Computes mean_n logsumexp_k [ -0.5*quad(n,k) + c_k ] over N=2M points,
K=32 components, D=16 dims, data-parallel over 8 NeuronCores.

Math:  quad(n,k) = |M_k x_n - M_k mu_k|^2 with M_k = chol(A_k A_k^T)^{-1},
       c_k = -logdet_k - (D/2) log 2pi + w_k^2.
Device computes, per point, z = G^T x~  (x~ = [x;1], G in [17, 512]),
quad_k = sum_i z_{k,i}^2, s = sum_k W_k exp(-quad_k/2)  (W_k = e^{c_k+S}),
and accumulates log s.  Host finishes: mean = (sum log s - pads)/N - S.

Device layout: points are split into 3 groups; group g's transposed
x~^T block occupies SBUF/DRAM partitions 32g..32g+16 (matmul operands
must share a base partition in {0,32,64}).  G is replicated at the same
three bases.  Each 128-point tile is two f32r matmuls (256 free cols
each -> one PSUM bank each), squared into fp16 (ScalarE mostly, DVE for
1 in 4 tiles), group-reduced on DVE to quad[128,32], then batched
exp/weight/sum and a final batched Ln with accumulate.
"""

from contextlib import ExitStack

import numpy as np

import concourse.bass as bass
import concourse.mybir as mybir
import concourse.tile as tile
from concourse import bacc
from concourse.bass_utils import run_bass_kernel_spmd

F32 = mybir.dt.float32
F32R = mybir.dt.float32r
F16 = mybir.dt.float16

# Problem constants
N_TOTAL = 2_000_000
D = 16
K = 32
NCORES = 8
NC = N_TOTAL // NCORES  # 250_000 points per core
GROUPS = 3              # point groups at SBUF base partitions 0/32/64
CPART = D + 1           # matmul contraction: 16 features + ones row
LOG_2PI = float(np.log(2.0 * np.pi))
SHIFT = 23.0            # folded into W so that log s lands near 0

# Full-size tiling (per core)
TPG = 656               # 128-point tiles per group
NG = TPG * 128          # padded points per group = 83_968
NPC = GROUPS * NG       # padded points per core = 251_904
WCHUNK = 2048           # DMA chunk columns; NG % WCHUNK == 0 (41 chunks)
TB = 24                 # tiles per exp batch; (GROUPS*TPG) % TB == 0

_MODULE_CACHE: dict = {}


def build_module(tpg: int = TPG, wchunk: int = WCHUNK, tb: int = TB,
                 dve_square_every: int = 10 ** 9, quad_dtype=F32,
                 reps: int = 1, gsize: int = 4, folds: int = 2):
    """Build the Bass module. Returns nc.

    Device I/O (per core):
      t    [51, tpg*128]  f32  input  (x~^T, 3 groups of 17 rows)
      g    [17, 512]      f32  input  (G cols, all K)
      wrep [128, K]       f32  input  (W_k replicated per partition)
      out  [128, 1]       f32  output (sum over tiles of log s)
    """
    ng = tpg * 128
    assert ng % wchunk == 0
    nchunks = ng // wchunk
    tiles_pcg = wchunk // 128
    ntiles = GROUPS * tpg
    assert ntiles % tb == 0 and tb % gsize == 0 and tiles_pcg % gsize == 0

    nc = bacc.Bacc("TRN2", target_bir_lowering=False, debug=False)

    t_in = nc.dram_tensor("t", [GROUPS * CPART, ng], F32R,
                          kind="ExternalInput").ap()
    g_in = nc.dram_tensor("g", [CPART, 2 * 256], F32R,
                          kind="ExternalInput").ap()
    w_in = nc.dram_tensor("wrep", [128, K], F32, kind="ExternalInput").ap()
    out = nc.dram_tensor("out", [128, 1], F32, kind="ExternalOutput").ap()

    AX = mybir.AxisListType
    OP = mybir.AluOpType
    AF = mybir.ActivationFunctionType

    with tile.TileContext(nc) as tc, ExitStack() as ctx:
        data_pool = ctx.enter_context(tc.tile_pool(name="data", bufs=2))
        zpool = ctx.enter_context(tc.tile_pool(name="z", bufs=8 // gsize, space="PSUM"))
        sqpool = ctx.enter_context(tc.tile_pool(name="sq", bufs=4))
        foldpool = ctx.enter_context(tc.tile_pool(name="fold", bufs=4))
        cpool = ctx.enter_context(tc.tile_pool(name="const", bufs=1))

        gt = cpool.tile([CPART, 2 * 256], F32R)
        nc.sync.dma_start(gt[:], g_in)
        wrept = cpool.tile([128, K], F32)
        nc.sync.dma_start(wrept[:], w_in)


        quad_buf = cpool.tile([128, 2, tb, K], quad_dtype)
        e_buf = cpool.tile([128, 2, tb, K], F32)
        ew_buf = cpool.tile([128, 2, tb, K], F32)
        s_buf = cpool.tile([128, ntiles], F32)
        ln_buf = cpool.tile([128, ntiles], F32)
        res = cpool.tile([128, 1], F32)

        wrep_bc = wrept[:].unsqueeze(1).broadcast_to([128, tb, K])

        def flush_batch(b):
            half = b % 2
            qv = quad_buf[:, half]          # [128, tb, K]
            ev = e_buf[:, half]
            ewv = ew_buf[:, half]
            nc.scalar.activation(ev, qv, AF.Exp, scale=-0.5)
            nc.vector.tensor_mul(ewv, ev, wrep_bc)
            nc.vector.tensor_reduce(
                s_buf[:, b * tb:(b + 1) * tb], ewv, axis=AX.X, op=OP.add)

        g0r = gt[:, 0:256]
        g1r = gt[:, 256:512]

        def emit_main():
            t_idx = 0
            for ch in range(nchunks):
                csl = slice(ch * wchunk, (ch + 1) * wchunk)
                dts = []
                for gb in range(GROUPS):
                    dt_g = data_pool.tile([CPART, wchunk], F32R,
                                          tag=f"dt{gb}")
                    nc.sync.dma_start(
                        dt_g[:], t_in[CPART * gb:CPART * (gb + 1), csl])
                    dts.append(dt_g)
                for g in range(GROUPS):
                    for j in range(0, tiles_pcg, gsize):
                        # gsize 128-point tiles -> gsize-bank PSUM tensor
                        zt = zpool.tile([128, gsize * 512], F32)
                        for u in range(gsize):
                            lhsT = dts[g][:, (j + u) * 128:(j + u + 1) * 128]
                            nc.tensor.matmul(zt[:, u * 512:u * 512 + 256],
                                             lhsT, g0r, start=True, stop=True)
                            nc.tensor.matmul(
                                zt[:, u * 512 + 256:u * 512 + 512],
                                lhsT, g1r, start=True, stop=True)
                        z3 = zt[:].rearrange("p (h c) -> p h c", h=2 * gsize)
                        sq = sqpool.tile([128, gsize * 512], F16)
                        sq3 = sq[:].rearrange("p (h c) -> p h c", h=2 * gsize)
                        if (t_idx // gsize) % dve_square_every == \
                                dve_square_every - 1:
                            # DVE path: TT may read only one PSUM operand,
                            # so round z to fp16 in SBUF first, then square
                            # at the 2x all-SBUF rate.
                            zc = foldpool.tile([128, gsize * 512], F16,
                                               tag="zc", name="zc")
                            nc.vector.tensor_copy(zc[:], zt[:])
                            zc3 = zc[:].rearrange("p (h c) -> p h c",
                                                  h=2 * gsize)
                            nc.vector.tensor_mul(sq3, zc3, zc3)
                        else:
                            nc.scalar.activation(sq3, z3, AF.Square)
                        b, tbi = divmod(t_idx, tb)
                        assert tbi + gsize <= tb
                        quad_t = quad_buf[:, b % 2, tbi:tbi + gsize]
                        with nc.allow_low_precision("quad rounding ok"):
                            red_in = sq[:].rearrange(
                                "p (t k i) -> p t k i", t=gsize, i=D)
                            w = D
                            for f in range(folds):
                                w //= 2
                                if w == 1:
                                    ftile = quad_t.unsqueeze(3)
                                else:
                                    fold_t = foldpool.tile(
                                        [128, gsize, K, w], F16,
                                        tag=f"fold{f}", name=f"fold{f}")
                                    ftile = fold_t[:]
                                nc.vector.tensor_add(
                                    ftile, red_in[:, :, :, 0:w],
                                    red_in[:, :, :, w:2 * w])
                                red_in = ftile
                            if w > 1:
                                nc.vector.tensor_reduce(
                                    quad_t, red_in, axis=AX.X, op=OP.add)
                        t_idx += gsize
                        if t_idx % tb == 0:
                            flush_batch(b)

        if reps == 1:
            emit_main()
        else:
            with tc.For_i(0, reps, 1):
                emit_main()

        nc.scalar.activation(ln_buf[:], s_buf[:], AF.Ln,
                             accum_out=res[:, 0:1])
        nc.sync.dma_start(out, res[:])

    if not nc.is_finalized():
        nc.finalize()
    return nc


def host_params(means, cov_parts, log_weights):
    """Fold model parameters into G (17 x 512), per-k weights, constants."""
    A = np.asarray(cov_parts, dtype=np.float64)
    mu = np.asarray(means, dtype=np.float64)
    w = np.asarray(log_weights, dtype=np.float64)

    cov = np.einsum('kij,klj->kil', A, A)
    L = np.linalg.cholesky(cov)
    eye = np.eye(D, dtype=np.float64)
    M = np.stack([np.linalg.solve(L[k], eye) for k in range(K)])  # [K, D, D]
    b = np.einsum('kij,kj->ki', M, mu)                            # [K, D]
    logdet = np.log(np.diagonal(L, axis1=1, axis2=2)).sum(axis=1)
    c = -0.5 * D * LOG_2PI - logdet + w ** 2                      # [K]

    G = np.zeros((CPART, K * D), dtype=np.float64)
    for k in range(K):
        cols = slice(k * D, (k + 1) * D)
        G[0:D, cols] = M[k].T          # col (k,i) rows 0..15 = M_k[i, :]
        G[D, cols] = -b[k]             # ones-row coefficient = -b_k[i]
    G = np.ascontiguousarray(G.astype(np.float32))

    W = np.exp(c + SHIFT).astype(np.float32)        # [K]
    wrep = np.ascontiguousarray(np.broadcast_to(W, (128, K))).astype(np.float32)

    # pad-point (x = 0) contribution: z = -b, quad = |b|^2
    lse_pad = np.log(np.sum(np.exp(c - 0.5 * (b ** 2).sum(axis=1))))
    logs0 = SHIFT + lse_pad
    return G, wrep, logs0


def build_t(data_core: np.ndarray, ng: int = NG) -> np.ndarray:
    """[npts, D] f32 -> [51, ng]: group g x~^T at rows 17g..17g+16."""
    npts = data_core.shape[0]
    npc = GROUPS * ng
    pad = npc - npts
    assert pad >= 0
    x = np.empty((npc, D), dtype=np.float32)
    x[:npts] = data_core
    if pad:
        x[npts:] = 0.0
    xg = x.reshape(GROUPS, ng, D)
    t = np.empty((GROUPS, CPART, ng), dtype=np.float32)
    t[:, :D, :] = xg.transpose(0, 2, 1)
    t[:, D, :] = 1.0
    return np.ascontiguousarray(t.reshape(GROUPS * CPART, ng))


def _get_module():
    key = (TPG, WCHUNK, TB)
    if key not in _MODULE_CACHE:
        _MODULE_CACHE[key] = build_module()
    return _MODULE_CACHE[key]


def run(data, means, cov_parts, log_weights, trace=False, **trace_kwargs):
    """Run on 8 cores; returns (answer_scalar, BassKernelResults)."""
    data = np.asarray(data)
    assert data.shape == (N_TOTAL, D), data.shape
    nc = _get_module()
    G, wrep, logs0 = host_params(means, cov_parts, log_weights)

    in_maps = []
    for core in range(NCORES):
        shard = data[core * NC:(core + 1) * NC]
        in_maps.append({"t": build_t(shard), "g": G, "wrep": wrep})
    res = run_bass_kernel_spmd(nc, in_maps, core_ids=list(range(NCORES)),
                               trace=trace, **trace_kwargs)

    total = 0.0
    for r in res.results:
        total += r["out"].astype(np.float64).sum()
    npad = NCORES * (NPC - NC)
    answer = (total - npad * logs0 - N_TOTAL * SHIFT) / N_TOTAL
    return np.float32(answer), res


def kernel(data, means, cov_parts, log_weights):
    ans, _ = run(data, means, cov_parts, log_weights, trace=False)
    return ans

